# revision 1
# baseline (speedup 1.0000x reference)
"""Trainium2 Bass kernel for nn_AttentionSiphon.

Reference computes: tokens = x @ W_map + b_map; concat [time, cluster, tokens];
LayerNorm; per-head q/k projections; softmax(q k^T / sqrt(dh)); mean over heads;
returns rows 0 and 1 of the [B,S,S] head-mean attention.

Only attention rows 0/1 are returned, and their queries come from the
(batch-independent) time/cluster tokens. So the per-head attention collapses:

  score[j, c=2h+r] = LN(token_j) . (Wk[h] @ q_r[h])   (+ constants)

which is one [D, 34] matmul against the token matrix (columns 32/33 gather the
LN statistics' linear parts). The heavy device work is the token projection
x @ W_map ([8184,512]@[512,1024]) plus the sum of squared tokens for the LN
variance, on 8 NeuronCores with rows sharded 1024/core (tail zero-padded).

Device output per core: [34, 2, 1024] — [:,0,:] holds 32 score columns plus
col-sum and b_map-cross rows, [0,1,:] the sumsq row. The tiny softmax
epilogue ([4,16,2,2048]) runs on host.
"""

import os
import sys

sys.path.insert(0, "/opt/trn_rl_repo")

import numpy as np
import ml_dtypes

B, N, IN_D = 4, 2046, 512
D, H, DH = 1024, 16, 64
S = N + 2
EPS = 1e-5
NCORES = 8
JPC = 1024            # padded rows per core
JTOT = NCORES * JPC   # 8192 (8184 real rows + 8 pad)
NAUG = 34             # 32 score cols + colsum + b_map cross
NC_OUT = 35           # + sumsq row

# Precision scheme: "f32r" (reduced-precision fp32 matmuls at bf16 speed,
# ~2e-4 rel err), "bf16" (fastest, ~3e-3), "split" (hi/lo bf16 3-pass, ~4e-6)
PRECISION = os.environ.get("AS_PRECISION", "f32r")
WARMUP_MMS = int(os.environ.get("AS_WARM", "38"))

_PROG_CACHE = {}
LAST_RESULT = None  # BassKernelResults of the most recent run (for test harness)


def _bf16(a):
    return np.asarray(a, np.float32).astype(ml_dtypes.bfloat16)


def _split_hi_lo(a):
    a = np.asarray(a, np.float32)
    hi = a.astype(ml_dtypes.bfloat16)
    lo = (a - hi.astype(np.float32)).astype(ml_dtypes.bfloat16)
    return hi, lo


def _build_program(precision, warmup=None):
    if warmup is None:
        warmup = WARMUP_MMS
    import concourse.bacc as bacc
    import concourse.mybir as mybir
    from concourse import tile
    from concourse.tile import ScopedClock

    class LeanTailTileContext(tile.TileContext):
        """Skip the exit-path double all-engine barrier + per-sem clears.

        The kernel preamble (Bass.__init__, target_bir_lowering) already
        dma_reset+sem_clears the kernel sem range at the start of every
        execution, and this program has a single TileContext, so nothing
        downstream consumes the freed sems. The final Sync drain still
        waits on every proc (incl. DMA lanes), so outputs are complete
        before the instruction streams end.
        """

        def _drain_and_barrier(self, tick_clock, wait_clock):
            drain_inst = self.nc.sync.drain()
            wait_clock.add_sem_waits(
                drain_inst.ins, ScopedClock({None: tick_clock.global_clock})
            )
            popped = self.nc._tile_sem_poison_stack.pop()
            assert popped is self._sem_poison

    f32 = mybir.dt.float32
    AF = mybir.ActivationFunctionType

    nc = bacc.Bacc("TRN2")

    split = precision == "split"
    # matmul operand dtype: bf16 for bf16/split schemes, f32r (full fp32
    # storage, reduced-precision PE matmul at bf16 speed) for "f32r"
    bf = mybir.dt.float32r if precision == "f32r" else mybir.dt.bfloat16
    bfsq = mybir.dt.bfloat16  # sumsq path: bf16 is plenty (var is benign)
    map_passes = [("hi", "hi"), ("hi", "lo"), ("lo", "hi")] if split else [("", "")]
    sfx = ("hi", "lo") if split else ("",)

    xt = {s: nc.dram_tensor(f"xt{s}", [128, 2, 4, 512], bf, kind="ExternalInput")
          for s in sfx}
    wm = {s: nc.dram_tensor(f"wm{s}", [128, 8, 4, 128], bf, kind="ExternalInput")
          for s in sfx}
    va = {s: nc.dram_tensor(f"va{s}", [128, 8, NAUG], bf, kind="ExternalInput")
          for s in sfx}
    # out[:, 0, :] = Y^T (32 scores + colsum + bcross); out[0, 1, :] = sumsq
    out_h = nc.dram_tensor("out", [NAUG, 2, JPC], f32, kind="ExternalOutput")

    NJT = JPC // 512  # 2 j-tiles

    with LeanTailTileContext(nc) as tc:
        with (
            tc.tile_pool(name="cst", bufs=1) as cst,
            tc.tile_pool(name="big", bufs=1) as big,
            tc.tile_pool(name="ps_map", bufs=4, space="PSUM") as ps_map,
            tc.tile_pool(name="ps_sml", bufs=2, space="PSUM") as ps_sml,
        ):
            xt_sb = {s: big.tile([128, 2, 4, 512], bf, name=f"xt{s}_sb", tag=f"xt{s}") for s in sfx}
            wm_sb = {s: big.tile([128, 8, 4, 128], bf, name=f"wm{s}_sb", tag=f"wm{s}") for s in sfx}
            va_sb = {s: cst.tile([128, 8, NAUG], bf, name=f"va{s}_sb", tag=f"va{s}") for s in sfx}
            tb_sb = {s: big.tile([128, 8, JPC], bf, name=f"tb{s}_sb", tag=f"tb{s}") for s in sfx}
            sq_sb = {s: big.tile([128, 8, JPC], bfsq, name=f"sq{s}_sb", tag=f"sq{s}") for s in sfx}
            mo_sb_f = cst.tile([128, 1], bfsq, name="mo_sb")  # ones column
            out_sb = cst.tile([NAUG, 2, JPC], f32)

            # Few DMA instructions (each costs ~650ns of sequencer issue
            # time), critical-path data first, all on one ring so the
            # early transfers aren't bandwidth-shared with the bulk.
            for s in sfx:
                nc.sync.dma_start(wm_sb[s][:, 0:2], wm[s][:, 0:2])
                nc.sync.dma_start(xt_sb[s][:, 0], xt[s][:, 0])
            for s in sfx:
                nc.sync.dma_start(wm_sb[s][:, 2:8], wm[s][:, 2:8])
            for s in sfx:
                nc.sync.dma_start(xt_sb[s][:, 1], xt[s][:, 1])
                nc.sync.dma_start(va_sb[s][:], va[s][:])
            nc.vector.memset(mo_sb_f[:], 1.0)
            nc.vector.memset(out_sb[:, 1, :], 0.0)
            mo_sb = mo_sb_f

            # PE warm-up during the DMA fill: dependency-free matmuls keep
            # the HAM activity monitor busy so the real matmuls start at
            # 2.4 GHz instead of the 1.2 GHz cold clock.
            if warmup:
                warm_sb = cst.tile([128, 256], mybir.dt.bfloat16,
                                   name="warm_sb")
                nc.vector.memset(warm_sb[:], 0.25)
                psw = ps_map.tile([128, 512], f32, name="psmap", tag="psmap")
                for w in range(warmup):
                    nc.tensor.matmul(
                        psw[:, 0:256], warm_sb[:, 0:128], warm_sb[:],
                        start=True, stop=True,
                    )

            # ---- token projection: T^T[d, j] = sum_i W[i,d] * xT[i,j] ----
            for jt in range(NJT):
                for dc in range(8):
                    ps = ps_map.tile([128, 512], f32, name="psmap", tag="psmap")
                    nmm = len(map_passes) * 4
                    k = 0
                    for (ls, rs) in map_passes:
                        for i in range(4):
                            nc.tensor.matmul(
                                ps[:],
                                wm_sb[ls][:, dc, i, :],
                                xt_sb[rs][:, jt, i, :],
                                start=(k == 0),
                                stop=(k == nmm - 1),
                            )
                            k += 1
                    jsl = slice(jt * 512, (jt + 1) * 512)
                    if split:
                        nc.vector.tensor_copy(tb_sb["hi"][:, dc, jsl], ps[:])
                        nc.vector.tensor_sub(
                            tb_sb["lo"][:, dc, jsl], ps[:], tb_sb["hi"][:, dc, jsl]
                        )
                        sqf = big.tile([128, 512], f32, name="sqf",
                                       tag="sqf", bufs=16)
                        nc.scalar.activation(sqf[:], ps[:], AF.Square)
                        nc.scalar.activation(
                            sq_sb["hi"][:, dc, jsl], sqf[:], AF.Copy
                        )
                        nc.vector.tensor_sub(
                            sq_sb["lo"][:, dc, jsl], sqf[:],
                            sq_sb["hi"][:, dc, jsl],
                        )
                    else:
                        nc.vector.tensor_copy(tb_sb[""][:, dc, jsl], ps[:])
                        nc.scalar.activation(
                            sq_sb[""][:, dc, jsl], ps[:], AF.Square
                        )

            # ---- scores/stats via Vaug^T @ T^T;  sumsq via ones^T @ SQ ----
            if split:
                sc_passes = [("hi", "hi"), ("hi", "lo"), ("lo", "hi")]
                sq_passes = [("", "hi"), ("", "lo")]
            else:
                sc_passes = [("", "")]
                sq_passes = [("", "")]
            for jt in range(NJT):
                jsl = slice(jt * 512, (jt + 1) * 512)
                psy = ps_sml.tile([NAUG, 512], f32, name="psy", tag="psy")
                nmm = len(sc_passes) * 8
                k = 0
                for (ls, rs) in sc_passes:
                    for dc in range(8):
                        nc.tensor.matmul(
                            psy[:],
                            va_sb[ls][:, dc, :],
                            tb_sb[rs][:, dc, jsl],
                            start=(k == 0),
                            stop=(k == nmm - 1),
                        )
                        k += 1
                nc.vector.tensor_copy(out_sb[0:NAUG, 0, jsl], psy[:])
                # ship the Y half now; the sumsq half follows while this
                # transfer drains (2D APs, one region per DMA)
                nc.sync.dma_start(out_h[:, 0, jsl], out_sb[:, 0, jsl])

                pss = ps_sml.tile([1, 512], f32, name="pss", tag="pss")
                nmm = len(sq_passes) * 8
                k = 0
                for (_, rs) in sq_passes:
                    for dc in range(8):
                        nc.tensor.matmul(
                            pss[:],
                            mo_sb[:],
                            sq_sb[rs][:, dc, jsl],
                            start=(k == 0),
                            stop=(k == nmm - 1),
                        )
                        k += 1
                nc.vector.tensor_copy(out_sb[0:1, 1, jsl], pss[:])
                nc.sync.dma_start(out_h[0:1, 1, jsl], out_sb[0:1, 1, jsl])

    nc.compile()
    return nc


def _host_precompute(inputs):
    x = np.asarray(inputs["x"], np.float32)
    W = np.asarray(inputs["W_map"], np.float32)
    b_map = np.asarray(inputs["b_map"], np.float32)
    g = np.asarray(inputs["ln_g"], np.float32)
    lb = np.asarray(inputs["ln_b"], np.float32)
    Wq = np.asarray(inputs["Wq"], np.float32)
    bq = np.asarray(inputs["bq"], np.float32)
    Wk = np.asarray(inputs["Wk"], np.float32)
    bk = np.asarray(inputs["bk"], np.float32)
    tt = np.asarray(inputs["time_token"], np.float32)
    ct = np.asarray(inputs["cluster_token"], np.float32)

    spec = np.concatenate([tt, ct], 0)                      # [2, D]
    mu = spec.mean(-1, keepdims=True)
    var = ((spec - mu) ** 2).mean(-1, keepdims=True)
    hspec = ((spec - mu) / np.sqrt(var + EPS) * g + lb).reshape(2, H, DH)
    q = np.einsum("rhd,hde->rhe", hspec, Wq) + bq[None]
    qs = (q / np.sqrt(DH)).astype(np.float32)               # [2,H,DH]
    kspec = np.einsum("rhd,hde->rhe", hspec, Wk) + bk[None]
    s_spec = np.einsum("rhe,the->hrt", qs, kspec)           # [H,2,2]

    v = np.einsum("hde,rhe->hdr", Wk, qs)                   # [H,DH,2]
    V = np.zeros((D, 2 * H), np.float32)
    for h in range(H):
        V[64 * h:64 * h + 64, 2 * h] = v[h, :, 0]
        V[64 * h:64 * h + 64, 2 * h + 1] = v[h, :, 1]
    c0 = np.empty(2 * H, np.float32)
    for h in range(H):
        c0[2 * h] = qs[0, h] @ bk[h]
        c0[2 * h + 1] = qs[1, h] @ bk[h]

    Vg = g[:, None] * V
    # augmented score matrix: [Vg | ones | b_map]
    Vaug = np.concatenate(
        [Vg, np.ones((D, 1), np.float32), b_map[:, None]], 1)  # [D, 34]
    consts = dict(
        pg=Vg.sum(0),
        qb=(lb[:, None] * V).sum(0),
        bVg=(b_map[:, None] * Vg).sum(0),
        bmean=b_map.mean(),
        bsq=(b_map ** 2).sum(),
        s_spec=s_spec,
        c0=c0,
    )
    return x, Vaug, W, consts


def kernel(**inputs):
    from concourse.bass_utils import run_bass_kernel_spmd

    x, Vaug, W, consts = _host_precompute(inputs)

    key = (PRECISION, WARMUP_MMS)
    if key not in _PROG_CACHE:
        _PROG_CACHE[key] = _build_program(PRECISION, WARMUP_MMS)
    nc = _PROG_CACHE[key]

    split = PRECISION == "split"

    xf = x.reshape(B * N, IN_D)
    xpad = np.zeros((JTOT, IN_D), np.float32)
    xpad[:B * N] = xf

    def pmaj(a, k):
        # [k*128, n] -> partition-major [128, k, n]
        return np.ascontiguousarray(
            np.asarray(a).reshape(k, 128, -1).transpose(1, 0, 2))

    def pack_wm(a):
        # [512, 1024] -> [128p, 8dc, 4i, 128]
        return np.ascontiguousarray(
            np.asarray(a).reshape(4, 128, 8, 128).transpose(1, 2, 0, 3))

    def pack_xt(a):
        # [512, 1024] -> [128p, 2jt, 4i, 512]
        return np.ascontiguousarray(
            np.asarray(a).reshape(4, 128, 2, 512).transpose(1, 2, 0, 3))

    shared = {}
    if split:
        Whi, Wlo = _split_hi_lo(W)
        Vhi, Vlo = _split_hi_lo(Vaug)
        shared["wmhi"] = pack_wm(Whi)
        shared["wmlo"] = pack_wm(Wlo)
        shared["vahi"] = pmaj(Vhi, 8)
        shared["valo"] = pmaj(Vlo, 8)
    elif PRECISION == "f32r":
        shared["wm"] = pack_wm(W)
        shared["va"] = pmaj(Vaug, 8)
    else:
        shared["wm"] = pack_wm(_bf16(W))
        shared["va"] = pmaj(_bf16(Vaug), 8)

    in_maps = []
    for c in range(NCORES):
        xT = np.ascontiguousarray(xpad[c * JPC:(c + 1) * JPC].T)  # [512, 1024]
        m = dict(shared)
        if split:
            xh, xl = _split_hi_lo(xT)
            m["xthi"] = pack_xt(xh)
            m["xtlo"] = pack_xt(xl)
        elif PRECISION == "f32r":
            m["xt"] = pack_xt(xT)
        else:
            m["xt"] = pack_xt(_bf16(xT))
        in_maps.append(m)

    trace = bool(int(os.environ.get("AS_TRACE", "0")))
    res = run_bass_kernel_spmd(nc, in_maps, list(range(NCORES)), trace=trace)
    global LAST_RESULT
    LAST_RESULT = res
    outs = [np.asarray(r["out"], np.float32) for r in res.results]

    return _epilogue(outs, consts)


def _epilogue(outs, consts):
    # outs: per-core [34, 2, JPC]; [:,0,:] = Y^T, [0,1,:] = sumsq
    yfull = np.concatenate([o[:, 0, :].T for o in outs], 0)[:B * N]
    SQ = np.concatenate([o[0, 1, :] for o in outs], 0)[:B * N]
    Y = yfull[:, 0:32]
    colsum = yfull[:, 32]
    bcross = yfull[:, 33]

    mu = colsum / np.float32(D) + consts["bmean"]
    E2 = (SQ + 2.0 * bcross + consts["bsq"]) / np.float32(D)
    var = E2 - mu ** 2
    rstd = (1.0 / np.sqrt(var + EPS)).astype(np.float32)
    G = Y + consts["bVg"][None]
    sc = (rstd[:, None] * G
          - (rstd * mu)[:, None] * consts["pg"][None]
          + consts["qb"][None] + consts["c0"][None])
    sc = sc.reshape(B, N, H, 2).transpose(0, 2, 3, 1)       # [B,H,2,N]

    scores = np.empty((B, H, 2, S), np.float32)
    scores[:, :, :, 2:] = sc
    scores[:, :, :, 0:2] = consts["s_spec"][None]

    m = scores - scores.max(-1, keepdims=True)
    e = np.exp(m)
    attn = e / e.sum(-1, keepdims=True)
    mm = attn.mean(1)                                       # [B,2,S]
    return (np.ascontiguousarray(mm[:, 0, :]),
            np.ascontiguousarray(mm[:, 1, :]))



# revision 8
# speedup vs baseline: 1.2263x; 1.2263x over previous
"""Trainium2 Bass kernel for nn_AttentionSiphon.

Reference computes: tokens = x @ W_map + b_map; concat [time, cluster, tokens];
LayerNorm; per-head q/k projections; softmax(q k^T / sqrt(dh)); mean over heads;
returns rows 0 and 1 of the [B,S,S] head-mean attention.

Only attention rows 0/1 are returned, and their queries come from the
(batch-independent) time/cluster tokens, so per-head attention collapses to

  score[j, c=2h+r] = LN(token_j) . (Wk[h] @ q_r[h])   (+ constants)

The 34 score/stat columns are LINEAR in x:  Y = Vaug^T (W^T x^T) = A^T x^T
with A = W @ Vaug [512, 34] precomputed on host.  Only the LayerNorm
sum-of-squares is quadratic:  SQ_j = ||W^T x_j||^2 = x_j^T (W W^T) x_j
= ||L^T x_j||^2 with L = cholesky(W W^T) [512, 512].  So the device work per
core (1024 token columns) is U = L^T x (512-contraction, half the FLOPs of the
naive 1024-wide token projection), squares+reduce for SQ, and the tiny A^T x.

Device output per core: [34, 2, 1024] f32 — [:,0,:] = Y^T, [0,1,:] = SQ.
The tiny softmax epilogue runs on host (identical to the previous scheme).
"""

import os
import sys

sys.path.insert(0, "/opt/trn_rl_repo")

import numpy as np
import ml_dtypes

B, N, IN_D = 4, 2046, 512
D, H, DH = 1024, 16, 64
S = N + 2
EPS = 1e-5
NCORES = 8
JPC = 1024            # padded rows per core
JTOT = NCORES * JPC   # 8192 (8184 real rows + 8 pad)
NAUG = 34             # 32 score cols + colsum + b_map cross
NC_OUT = NAUG + 1     # + sumsq row

# Precision scheme: "bf16" (fastest, ~1.6e-3 rel err),
# "f32r" (fp32-storage reduced-precision matmuls at bf16 PE speed, ~2e-4)
PRECISION = os.environ.get("AS_PRECISION", "bf16")
WARMUP_MMS = int(os.environ.get("AS_WARM", "60"))

_PROG_CACHE = {}
LAST_RESULT = None  # BassKernelResults of the most recent run (for test harness)


def _bf16(a):
    return np.asarray(a, np.float32).astype(ml_dtypes.bfloat16)


def _build_program(precision, warmup=None):
    if warmup is None:
        warmup = WARMUP_MMS
    import concourse.bacc as bacc
    import concourse.mybir as mybir
    from concourse import tile
    from concourse.tile import ScopedClock

    class LeanTailTileContext(tile.TileContext):
        """Skip the exit-path double all-engine barrier + per-sem clears.

        The kernel preamble (Bass.__init__, target_bir_lowering) already
        dma_reset+sem_clears the kernel sem range at the start of every
        execution, and this program has a single TileContext, so nothing
        downstream consumes the freed sems. The final Sync drain still
        waits on every proc (incl. DMA lanes), so outputs are complete
        before the instruction streams end.
        """

        def _drain_and_barrier(self, tick_clock, wait_clock):
            drain_inst = self.nc.sync.drain()
            wait_clock.add_sem_waits(
                drain_inst.ins, ScopedClock({None: tick_clock.global_clock})
            )
            popped = self.nc._tile_sem_poison_stack.pop()
            assert popped is self._sem_poison

    f32 = mybir.dt.float32
    bf16 = mybir.dt.bfloat16
    AF = mybir.ActivationFunctionType

    nc = bacc.Bacc("TRN2")

    bf = mybir.dt.float32r if precision == "f32r" else bf16

    xt = nc.dram_tensor("xt", [128, 2, 4, 512], bf, kind="ExternalInput")
    lw = nc.dram_tensor("lw", [128, 4, 4, 128], bf, kind="ExternalInput")
    va = nc.dram_tensor("va", [128, 4, NAUG], bf, kind="ExternalInput")
    # out[:, 0, :] = Y^T (32 scores + colsum + bcross); out[0, 1, :] = sumsq
    # (sumsq lives in plane 1 at partition 0 — engines can't copy across
    # partitions, and the [1, 512] ones-matmul result sits at partition 0)
    out_h = nc.dram_tensor("out", [NAUG, 2, JPC], f32, kind="ExternalOutput")

    ones_bf = nc.const_aps.tensor(1.0, [128, 1], bf16)

    with LeanTailTileContext(nc) as tc:
        with (
            tc.tile_pool(name="cst", bufs=1) as cst,
            tc.tile_pool(name="scr", bufs=2) as scr,
            tc.tile_pool(name="ps_u", bufs=4, space="PSUM") as ps_u,
            tc.tile_pool(name="ps_y", bufs=1, space="PSUM") as ps_y,
            tc.tile_pool(name="ps_s", bufs=1, space="PSUM") as ps_s,
        ):
            xt_sb = cst.tile([128, 2, 4, 512], bf, name="xt_sb", tag="xt")
            lw_sb = cst.tile([128, 4, 4, 128], bf, name="lw_sb", tag="lw")
            va_sb = cst.tile([128, 4, NAUG], bf, name="va_sb", tag="va")
            out_sb = cst.tile([NAUG, 2, JPC], f32, name="out_sb")

            # Input DMA: two issue queues (Sync + Scalar HWDGE) so issue
            # latency overlaps; everything needed by jt0 goes first.
            nc.sync.dma_start(lw_sb[:], lw[:])
            nc.sync.dma_start(va_sb[:], va[:])
            nc.scalar.dma_start(xt_sb[:, 0], xt[:, 0])
            nc.scalar.dma_start(xt_sb[:, 1], xt[:, 1])

            # PE warm-up during the DMA fill: tiny dependency-free matmuls
            # off the (preamble-initialized) const AP keep the HAM activity
            # monitor busy so real matmuls start at 2.4 GHz. N=1 each, so
            # issue-rate-bound (~50ns) with no memset dependency.
            if warmup:
                psw = ps_s.tile([1, 512], f32, name="psw", tag="pss")
                for _ in range(warmup):
                    nc.tensor.matmul(psw[0:1, 0:1], ones_bf, ones_bf,
                                     start=True, stop=True)

            for jt in range(2):
                jsl = slice(jt * 512, (jt + 1) * 512)
                # ---- U = L^T x ---- 4 output chunks of 128 d-rows
                sq = []
                for dc in range(4):
                    psu = ps_u.tile([128, 512], f32, name="psu", tag="psu")
                    for kc in range(4):
                        nc.tensor.matmul(
                            psu[:],
                            lw_sb[:, kc, dc, :],
                            xt_sb[:, jt, kc, :],
                            start=(kc == 0),
                            stop=(kc == 3),
                        )
                    # squared chunk (bf16; LN variance is error-tolerant)
                    sq_t = scr.tile([128, 512], bf16, name=f"sq{dc}",
                                    tag=f"sq{dc}")
                    nc.scalar.activation(sq_t[:], psu[:], AF.Square)
                    sq.append(sq_t)

                # ---- scores Y^T = A^T x ----
                psy = ps_y.tile([NAUG, 512], f32, name="psy", tag="psy")
                for kc in range(4):
                    nc.tensor.matmul(
                        psy[:],
                        va_sb[:, kc, :],
                        xt_sb[:, jt, kc, :],
                        start=(kc == 0),
                        stop=(kc == 3),
                    )
                nc.vector.tensor_copy(out_sb[:, 0, jsl], psy[:])
                nc.sync.dma_start(out_h[:, 0, jsl], out_sb[:, 0, jsl])

                # ---- SQ: fold 4 chunks (vector), then ones-matmul ----
                s01 = scr.tile([128, 512], bf16, name="s01", tag="s01")
                s23 = scr.tile([128, 512], bf16, name="s23", tag="s23")
                sfin = scr.tile([128, 512], bf16, name="sfin", tag="sfin")
                nc.vector.tensor_add(s01[:], sq[0][:], sq[1][:])
                nc.vector.tensor_add(s23[:], sq[2][:], sq[3][:])
                nc.vector.tensor_add(sfin[:], s01[:], s23[:])
                pss = ps_s.tile([1, 512], f32, name="pss", tag="pss")
                nc.tensor.matmul(pss[:], ones_bf, sfin[:],
                                 start=True, stop=True)
                nc.vector.tensor_copy(out_sb[0:1, 1, jsl], pss[:])
                nc.sync.dma_start(out_h[0:1, 1, jsl], out_sb[0:1, 1, jsl])

    nc.compile()
    return nc


def _host_precompute(inputs):
    x = np.asarray(inputs["x"], np.float32)
    W = np.asarray(inputs["W_map"], np.float32)
    b_map = np.asarray(inputs["b_map"], np.float32)
    g = np.asarray(inputs["ln_g"], np.float32)
    lb = np.asarray(inputs["ln_b"], np.float32)
    Wq = np.asarray(inputs["Wq"], np.float32)
    bq = np.asarray(inputs["bq"], np.float32)
    Wk = np.asarray(inputs["Wk"], np.float32)
    bk = np.asarray(inputs["bk"], np.float32)
    tt = np.asarray(inputs["time_token"], np.float32)
    ct = np.asarray(inputs["cluster_token"], np.float32)

    spec = np.concatenate([tt, ct], 0)                      # [2, D]
    mu = spec.mean(-1, keepdims=True)
    var = ((spec - mu) ** 2).mean(-1, keepdims=True)
    hspec = ((spec - mu) / np.sqrt(var + EPS) * g + lb).reshape(2, H, DH)
    q = np.einsum("rhd,hde->rhe", hspec, Wq) + bq[None]
    qs = (q / np.sqrt(DH)).astype(np.float32)               # [2,H,DH]
    kspec = np.einsum("rhd,hde->rhe", hspec, Wk) + bk[None]
    s_spec = np.einsum("rhe,the->hrt", qs, kspec)           # [H,2,2]

    v = np.einsum("hde,rhe->hdr", Wk, qs)                   # [H,DH,2]
    V = np.zeros((D, 2 * H), np.float32)
    for h in range(H):
        V[64 * h:64 * h + 64, 2 * h] = v[h, :, 0]
        V[64 * h:64 * h + 64, 2 * h + 1] = v[h, :, 1]
    c0 = np.empty(2 * H, np.float32)
    for h in range(H):
        c0[2 * h] = qs[0, h] @ bk[h]
        c0[2 * h + 1] = qs[1, h] @ bk[h]

    Vg = g[:, None] * V
    # augmented score matrix: [Vg | ones | b_map]
    Vaug = np.concatenate(
        [Vg, np.ones((D, 1), np.float32), b_map[:, None]], 1)  # [D, 34]
    consts = dict(
        pg=Vg.sum(0),
        qb=(lb[:, None] * V).sum(0),
        bVg=(b_map[:, None] * Vg).sum(0),
        bmean=b_map.mean(),
        bsq=(b_map ** 2).sum(),
        s_spec=s_spec,
        c0=c0,
    )

    # collapse the linear part through W; factor the quadratic part
    W64 = W.astype(np.float64)
    A = (W64 @ Vaug.astype(np.float64)).astype(np.float32)  # [512, 34]
    L = np.linalg.cholesky(W64 @ W64.T).astype(np.float32)  # [512, 512]
    return x, A, L, consts


def kernel(**inputs):
    from concourse.bass_utils import run_bass_kernel_spmd

    x, A, L, consts = _host_precompute(inputs)

    key = (PRECISION, WARMUP_MMS)
    if key not in _PROG_CACHE:
        _PROG_CACHE[key] = _build_program(PRECISION, WARMUP_MMS)
    nc = _PROG_CACHE[key]

    cast = (lambda a: np.asarray(a, np.float32)) if PRECISION == "f32r" \
        else _bf16

    xf = x.reshape(B * N, IN_D)
    xpad = np.zeros((JTOT, IN_D), np.float32)
    xpad[:B * N] = xf

    # [512, 512] -> [128p, 4kc, 4dc, 128]
    lw_p = np.ascontiguousarray(
        cast(L).reshape(4, 128, 4, 128).transpose(1, 0, 2, 3))
    # [512, 34] -> [128p, 4kc, 34]
    va_p = np.ascontiguousarray(
        cast(A).reshape(4, 128, NAUG).transpose(1, 0, 2))
    shared = {"lw": lw_p, "va": va_p}

    in_maps = []
    for c in range(NCORES):
        xT = np.ascontiguousarray(xpad[c * JPC:(c + 1) * JPC].T)  # [512,1024]
        m = dict(shared)
        # [512, 1024] -> [128p, 2jt, 4kc, 512]
        m["xt"] = np.ascontiguousarray(
            cast(xT).reshape(4, 128, 2, 512).transpose(1, 2, 0, 3))
        in_maps.append(m)

    trace = bool(int(os.environ.get("AS_TRACE", "0")))
    res = run_bass_kernel_spmd(nc, in_maps, list(range(NCORES)), trace=trace)
    global LAST_RESULT
    LAST_RESULT = res
    outs = [np.asarray(r["out"], np.float32) for r in res.results]

    return _epilogue(outs, consts)


def _epilogue(outs, consts):
    # outs: per-core [34, 2, JPC]; [:,0,:] = Y^T, [0,1,:] = SQ
    yfull = np.concatenate([o[:, 0, :].T for o in outs], 0)[:B * N]
    SQ = np.concatenate([o[0, 1, :] for o in outs], 0)[:B * N]
    Y = yfull[:, 0:32]
    colsum = yfull[:, 32]
    bcross = yfull[:, 33]

    mu = colsum / np.float32(D) + consts["bmean"]
    E2 = (SQ + 2.0 * bcross + consts["bsq"]) / np.float32(D)
    var = E2 - mu ** 2
    rstd = (1.0 / np.sqrt(var + EPS)).astype(np.float32)
    G = Y + consts["bVg"][None]
    sc = (rstd[:, None] * G
          - (rstd * mu)[:, None] * consts["pg"][None]
          + consts["qb"][None] + consts["c0"][None])
    sc = sc.reshape(B, N, H, 2).transpose(0, 2, 3, 1)       # [B,H,2,N]

    scores = np.empty((B, H, 2, S), np.float32)
    scores[:, :, :, 2:] = sc
    scores[:, :, :, 0:2] = consts["s_spec"][None]

    m = scores - scores.max(-1, keepdims=True)
    e = np.exp(m)
    attn = e / e.sum(-1, keepdims=True)
    mm = attn.mean(1)                                       # [B,2,S]
    return (np.ascontiguousarray(mm[:, 0, :]),
            np.ascontiguousarray(mm[:, 1, :]))


# revision 12
# speedup vs baseline: 1.3837x; 1.1283x over previous
"""Trainium2 Bass kernel for nn_AttentionSiphon.

Reference computes: tokens = x @ W_map + b_map; concat [time, cluster, tokens];
LayerNorm; per-head q/k projections; softmax(q k^T / sqrt(dh)); mean over heads;
returns rows 0 and 1 of the [B,S,S] head-mean attention.

Only attention rows 0/1 are returned, and their queries come from the
(batch-independent) time/cluster tokens, so per-head attention collapses to

  score[j, c=2h+r] = LN(token_j) . (Wk[h] @ q_r[h])   (+ constants)

The 34 score/stat columns are LINEAR in x:  Y = Vaug^T (W^T x^T) = A^T x^T
with A = W @ Vaug [512, 34] precomputed on host.  Only the LayerNorm
sum-of-squares is quadratic:  SQ_j = ||W^T x_j||^2 = x_j^T (W W^T) x_j
= ||L^T x_j||^2 with L = cholesky(W W^T) [512, 512].  So the device work per
core (1024 token columns) is U = L^T x (512-contraction, half the FLOPs of the
naive 1024-wide token projection), squares+reduce for SQ, and the tiny A^T x.
L is lower-triangular, so of the 4x4 grid of [128,128] contraction blocks only
the kc >= dc ones are nonzero: 10 matmuls per 512-column tile instead of 16.

Device output per core: [34, 2, 1024] f32 — [:,0,:] = Y^T, [0,1,:] = SQ.
The tiny softmax epilogue runs on host (identical to the previous scheme).
"""

import os
import sys

sys.path.insert(0, "/opt/trn_rl_repo")

import numpy as np
import ml_dtypes

B, N, IN_D = 4, 2046, 512
D, H, DH = 1024, 16, 64
S = N + 2
EPS = 1e-5
NCORES = 8
JPC = 1024            # padded rows per core
JTOT = NCORES * JPC   # 8192 (8184 real rows + 8 pad)
NAUG = 34             # 32 score cols + colsum + b_map cross
NC_OUT = NAUG + 1     # + sumsq row

# Precision scheme: "bf16" (fastest, ~1.6e-3 rel err),
# "f32r" (fp32-storage reduced-precision matmuls at bf16 PE speed, ~2e-4)
PRECISION = os.environ.get("AS_PRECISION", "bf16")
WARMUP_MMS = int(os.environ.get("AS_WARM", "150"))

_PROG_CACHE = {}
LAST_RESULT = None  # BassKernelResults of the most recent run (for test harness)


def _bf16(a):
    return np.asarray(a, np.float32).astype(ml_dtypes.bfloat16)


def _build_program(precision, warmup=None):
    if warmup is None:
        warmup = WARMUP_MMS
    import concourse.bacc as bacc
    import concourse.mybir as mybir
    from concourse import tile
    from concourse.tile import ScopedClock

    class LeanTailTileContext(tile.TileContext):
        """Skip the exit-path double all-engine barrier + per-sem clears.

        The kernel preamble (Bass.__init__, target_bir_lowering) already
        dma_reset+sem_clears the kernel sem range at the start of every
        execution, and this program has a single TileContext, so nothing
        downstream consumes the freed sems. The final Sync drain still
        waits on every proc (incl. DMA lanes), so outputs are complete
        before the instruction streams end.
        """

        def _drain_and_barrier(self, tick_clock, wait_clock):
            drain_inst = self.nc.sync.drain()
            wait_clock.add_sem_waits(
                drain_inst.ins, ScopedClock({None: tick_clock.global_clock})
            )
            popped = self.nc._tile_sem_poison_stack.pop()
            assert popped is self._sem_poison

    f32 = mybir.dt.float32
    bf16 = mybir.dt.bfloat16
    AF = mybir.ActivationFunctionType

    nc = bacc.Bacc("TRN2")

    bf = mybir.dt.float32r if precision == "f32r" else bf16

    # L-blocks (kc>=dc, per dc in emission order dc=3,2,1,0) + A chunks,
    # all fused into one per-partition-contiguous tensor for a single
    # fat-packet DMA.  Column offsets precomputed here.
    DCS = [3, 2, 1, 0]
    lblk = {}
    col = 0
    for dc in DCS:
        for kc in range(dc, 4):
            lblk[(dc, kc)] = col
            col += 128
    acol = {}
    for kc in range(4):
        acol[kc] = col
        col += NAUG
    LWA_W = col  # 10*128 + 4*34 = 1416

    xt = nc.dram_tensor("xt", [128, 2, 4, 512], bf, kind="ExternalInput")
    lwa = nc.dram_tensor("lwa", [128, LWA_W], bf, kind="ExternalInput")
    # out[:, 0, :] = Y^T (32 scores + colsum + bcross); out[0, 1, :] = sumsq
    # (sumsq lives in plane 1 at partition 0 — engines can't copy across
    # partitions, and the [1, 512] ones-matmul result sits at partition 0)
    out_h = nc.dram_tensor("out", [NAUG, 2, JPC], f32, kind="ExternalOutput")

    ones_bf = nc.const_aps.tensor(1.0, [128, 1], bf16)

    with LeanTailTileContext(nc) as tc:
        with (
            tc.tile_pool(name="cst", bufs=1) as cst,
            tc.tile_pool(name="scr", bufs=2) as scr,
            tc.tile_pool(name="ps_u", bufs=4, space="PSUM") as ps_u,
            tc.tile_pool(name="ps_y", bufs=1, space="PSUM") as ps_y,
            tc.tile_pool(name="ps_s", bufs=1, space="PSUM") as ps_s,
        ):
            xt_sb = cst.tile([128, 2, 4, 512], bf, name="xt_sb", tag="xt")
            lwa_sb = cst.tile([128, LWA_W], bf, name="lwa_sb", tag="lwa")
            out_sb = cst.tile([NAUG, 2, JPC], f32, name="out_sb")

            # Input DMA on two issue queues (Sync + Scalar HWDGE); jt0's x
            # halves go first so the first U matmuls can start before the
            # full fill completes.
            nc.sync.dma_start(lwa_sb[:], lwa[:])
            nc.scalar.dma_start(xt_sb[:, 0, 0:2], xt[:, 0, 0:2])
            nc.scalar.dma_start(xt_sb[:, 0, 2:4], xt[:, 0, 2:4])
            nc.sync.dma_start(xt_sb[:, 1, 0:2], xt[:, 1, 0:2])
            nc.sync.dma_start(xt_sb[:, 1, 2:4], xt[:, 1, 2:4])

            # PE warm-up during the DMA fill: tiny dependency-free matmuls
            # off the (preamble-initialized) const AP keep the HAM activity
            # monitor busy so real matmuls start at 2.4 GHz. N=1 each
            # (~33ns issue rate), no memset dependency.
            if warmup:
                psw = ps_s.tile([1, 512], f32, name="psw", tag="psw")
                for _ in range(warmup):
                    nc.tensor.matmul(psw[0:1, 0:1], ones_bf, ones_bf,
                                     start=True, stop=True)

            for jt in range(2):
                jsl = slice(jt * 512, (jt + 1) * 512)
                # ---- U = L^T x (triangular: block dc needs kc>=dc) ----
                # dc=3 first (1 matmul) so its square lands early; the
                # sumsq ones-matmuls accumulate as squares become ready,
                # with Y before the last one so the PE never stalls.
                sq = {}
                for dc in DCS:
                    psu = ps_u.tile([128, 512], f32, name="psu", tag="psu")
                    kcs = list(range(dc, 4))
                    for ki, kc in enumerate(kcs):
                        nc.tensor.matmul(
                            psu[:],
                            lwa_sb[:, lblk[(dc, kc)]:lblk[(dc, kc)] + 128],
                            xt_sb[:, jt, kc, :],
                            start=(ki == 0),
                            stop=(ki == len(kcs) - 1),
                        )
                    # squared chunk (bf16; LN variance is error-tolerant)
                    sq_t = scr.tile([128, 512], bf16, name=f"sq{dc}",
                                    tag=f"sq{dc}")
                    nc.scalar.activation(sq_t[:], psu[:], AF.Square)
                    sq[dc] = sq_t

                # ---- sumsq: accumulate ones^T @ sq[dc] into one [1,512]
                pss = ps_s.tile([1, 512], f32, name="pss", tag="pss")
                for dc in [3, 2, 1]:
                    nc.tensor.matmul(pss[:], ones_bf, sq[dc][:],
                                     start=(dc == 3), stop=False)

                # ---- scores Y^T = A^T x ----
                psy = ps_y.tile([NAUG, 512], f32, name="psy", tag="psy")
                for kc in range(4):
                    nc.tensor.matmul(
                        psy[:],
                        lwa_sb[:, acol[kc]:acol[kc] + NAUG],
                        xt_sb[:, jt, kc, :],
                        start=(kc == 0),
                        stop=(kc == 3),
                    )
                # last sumsq chunk lands while Y streams; Y is the last
                # big PE op so its output ships immediately
                nc.tensor.matmul(pss[:], ones_bf, sq[0][:],
                                 start=False, stop=True)

                nc.vector.tensor_copy(out_sb[:, 0, jsl], psy[:])
                nc.sync.dma_start(out_h[:, 0, jsl], out_sb[:, 0, jsl])
                nc.vector.tensor_copy(out_sb[0:1, 1, jsl], pss[:])
                nc.sync.dma_start(out_h[0:1, 1, jsl], out_sb[0:1, 1, jsl])

    nc.compile()
    return nc


def _host_precompute(inputs):
    x = np.asarray(inputs["x"], np.float32)
    W = np.asarray(inputs["W_map"], np.float32)
    b_map = np.asarray(inputs["b_map"], np.float32)
    g = np.asarray(inputs["ln_g"], np.float32)
    lb = np.asarray(inputs["ln_b"], np.float32)
    Wq = np.asarray(inputs["Wq"], np.float32)
    bq = np.asarray(inputs["bq"], np.float32)
    Wk = np.asarray(inputs["Wk"], np.float32)
    bk = np.asarray(inputs["bk"], np.float32)
    tt = np.asarray(inputs["time_token"], np.float32)
    ct = np.asarray(inputs["cluster_token"], np.float32)

    spec = np.concatenate([tt, ct], 0)                      # [2, D]
    mu = spec.mean(-1, keepdims=True)
    var = ((spec - mu) ** 2).mean(-1, keepdims=True)
    hspec = ((spec - mu) / np.sqrt(var + EPS) * g + lb).reshape(2, H, DH)
    q = np.einsum("rhd,hde->rhe", hspec, Wq) + bq[None]
    qs = (q / np.sqrt(DH)).astype(np.float32)               # [2,H,DH]
    kspec = np.einsum("rhd,hde->rhe", hspec, Wk) + bk[None]
    s_spec = np.einsum("rhe,the->hrt", qs, kspec)           # [H,2,2]

    v = np.einsum("hde,rhe->hdr", Wk, qs)                   # [H,DH,2]
    V = np.zeros((D, 2 * H), np.float32)
    for h in range(H):
        V[64 * h:64 * h + 64, 2 * h] = v[h, :, 0]
        V[64 * h:64 * h + 64, 2 * h + 1] = v[h, :, 1]
    c0 = np.empty(2 * H, np.float32)
    for h in range(H):
        c0[2 * h] = qs[0, h] @ bk[h]
        c0[2 * h + 1] = qs[1, h] @ bk[h]

    Vg = g[:, None] * V
    # augmented score matrix: [Vg | ones | b_map]
    Vaug = np.concatenate(
        [Vg, np.ones((D, 1), np.float32), b_map[:, None]], 1)  # [D, 34]
    consts = dict(
        pg=Vg.sum(0),
        qb=(lb[:, None] * V).sum(0),
        bVg=(b_map[:, None] * Vg).sum(0),
        bmean=b_map.mean(),
        bsq=(b_map ** 2).sum(),
        s_spec=s_spec,
        c0=c0,
    )

    # collapse the linear part through W; factor the quadratic part
    W64 = W.astype(np.float64)
    A = (W64 @ Vaug.astype(np.float64)).astype(np.float32)  # [512, 34]
    L = np.linalg.cholesky(W64 @ W64.T).astype(np.float32)  # [512, 512]
    return x, A, L, consts


def kernel(**inputs):
    from concourse.bass_utils import run_bass_kernel_spmd

    x, A, L, consts = _host_precompute(inputs)

    key = (PRECISION, WARMUP_MMS)
    if key not in _PROG_CACHE:
        _PROG_CACHE[key] = _build_program(PRECISION, WARMUP_MMS)
    nc = _PROG_CACHE[key]

    cast = (lambda a: np.asarray(a, np.float32)) if PRECISION == "f32r" \
        else _bf16

    xf = x.reshape(B * N, IN_D)
    xpad = np.zeros((JTOT, IN_D), np.float32)
    xpad[:B * N] = xf

    # fused L-blocks (kc>=dc, dc order 3,2,1,0) + A chunks: [128, 1416]
    lwa_np = np.empty((128, 10 * 128 + 4 * NAUG), np.float32)
    col = 0
    for dc in [3, 2, 1, 0]:
        for kc in range(dc, 4):
            lwa_np[:, col:col + 128] = \
                L[kc * 128:(kc + 1) * 128, dc * 128:(dc + 1) * 128]
            col += 128
    for kc in range(4):
        lwa_np[:, col:col + NAUG] = A[kc * 128:(kc + 1) * 128, :]
        col += NAUG
    shared = {"lwa": np.ascontiguousarray(cast(lwa_np))}

    in_maps = []
    for c in range(NCORES):
        xT = np.ascontiguousarray(xpad[c * JPC:(c + 1) * JPC].T)  # [512,1024]
        m = dict(shared)
        # [512, 1024] -> [128p, 2jt, 4kc, 512]
        m["xt"] = np.ascontiguousarray(
            cast(xT).reshape(4, 128, 2, 512).transpose(1, 2, 0, 3))
        in_maps.append(m)

    trace = bool(int(os.environ.get("AS_TRACE", "0")))
    res = run_bass_kernel_spmd(nc, in_maps, list(range(NCORES)), trace=trace)
    global LAST_RESULT
    LAST_RESULT = res
    outs = [np.asarray(r["out"], np.float32) for r in res.results]

    return _epilogue(outs, consts)


def _epilogue(outs, consts):
    # outs: per-core [34, 2, JPC]; [:,0,:] = Y^T, [0,1,:] = SQ
    yfull = np.concatenate([o[:, 0, :].T for o in outs], 0)[:B * N]
    SQ = np.concatenate([o[0, 1, :] for o in outs], 0)[:B * N]
    Y = yfull[:, 0:32]
    colsum = yfull[:, 32]
    bcross = yfull[:, 33]

    mu = colsum / np.float32(D) + consts["bmean"]
    E2 = (SQ + 2.0 * bcross + consts["bsq"]) / np.float32(D)
    var = E2 - mu ** 2
    rstd = (1.0 / np.sqrt(var + EPS)).astype(np.float32)
    G = Y + consts["bVg"][None]
    sc = (rstd[:, None] * G
          - (rstd * mu)[:, None] * consts["pg"][None]
          + consts["qb"][None] + consts["c0"][None])
    sc = sc.reshape(B, N, H, 2).transpose(0, 2, 3, 1)       # [B,H,2,N]

    scores = np.empty((B, H, 2, S), np.float32)
    scores[:, :, :, 2:] = sc
    scores[:, :, :, 0:2] = consts["s_spec"][None]

    m = scores - scores.max(-1, keepdims=True)
    e = np.exp(m)
    attn = e / e.sum(-1, keepdims=True)
    mm = attn.mean(1)                                       # [B,2,S]
    return (np.ascontiguousarray(mm[:, 0, :]),
            np.ascontiguousarray(mm[:, 1, :]))


# revision 15
# speedup vs baseline: 1.3848x; 1.0008x over previous
"""Trainium2 Bass kernel for nn_AttentionSiphon.

Reference computes: tokens = x @ W_map + b_map; concat [time, cluster, tokens];
LayerNorm; per-head q/k projections; softmax(q k^T / sqrt(dh)); mean over heads;
returns rows 0 and 1 of the [B,S,S] head-mean attention.

Only attention rows 0/1 are returned, and their queries come from the
(batch-independent) time/cluster tokens, so per-head attention collapses to

  score[j, c=2h+r] = LN(token_j) . (Wk[h] @ q_r[h])   (+ constants)

The 34 score/stat columns are LINEAR in x:  Y = Vaug^T (W^T x^T) = A^T x^T
with A = W @ Vaug [512, 34] precomputed on host.  Only the LayerNorm
sum-of-squares is quadratic:  SQ_j = ||W^T x_j||^2 = x_j^T (W W^T) x_j
= ||L^T x_j||^2 with L = cholesky(W W^T) [512, 512].  So the device work per
core (1024 token columns) is U = L^T x (512-contraction, half the FLOPs of the
naive 1024-wide token projection), squares+reduce for SQ, and the tiny A^T x.
L is lower-triangular, so of the 4x4 grid of [128,128] contraction blocks only
the kc >= dc ones are nonzero: 10 matmuls per 512-column tile instead of 16.

Device output per core: [34, 2, 1024] f32 — [:,0,:] = Y^T, [0,1,:] = SQ.
The tiny softmax epilogue runs on host (identical to the previous scheme).
"""

import os
import sys

sys.path.insert(0, "/opt/trn_rl_repo")

import numpy as np
import ml_dtypes

B, N, IN_D = 4, 2046, 512
D, H, DH = 1024, 16, 64
S = N + 2
EPS = 1e-5
NCORES = 8
JPC = 1024            # padded rows per core
JTOT = NCORES * JPC   # 8192 (8184 real rows + 8 pad)
NAUG = 34             # 32 score cols + colsum + b_map cross
NC_OUT = NAUG + 1     # + sumsq row

# Precision scheme: "bf16" (fastest, ~1.6e-3 rel err),
# "f32r" (fp32-storage reduced-precision matmuls at bf16 PE speed, ~2e-4)
PRECISION = os.environ.get("AS_PRECISION", "bf16")
WARMUP_MMS = int(os.environ.get("AS_WARM", "16"))

_PROG_CACHE = {}
LAST_RESULT = None  # BassKernelResults of the most recent run (for test harness)


def _bf16(a):
    return np.asarray(a, np.float32).astype(ml_dtypes.bfloat16)


def _build_program(precision, warmup=None):
    if warmup is None:
        warmup = WARMUP_MMS
    import concourse.bacc as bacc
    import concourse.mybir as mybir
    from concourse import tile
    from concourse.tile import ScopedClock

    class LeanTailTileContext(tile.TileContext):
        """Skip the exit-path double all-engine barrier + per-sem clears.

        The kernel preamble (Bass.__init__, target_bir_lowering) already
        dma_reset+sem_clears the kernel sem range at the start of every
        execution, and this program has a single TileContext, so nothing
        downstream consumes the freed sems. The final Sync drain still
        waits on every proc (incl. DMA lanes), so outputs are complete
        before the instruction streams end.
        """

        def _drain_and_barrier(self, tick_clock, wait_clock):
            drain_inst = self.nc.sync.drain()
            wait_clock.add_sem_waits(
                drain_inst.ins, ScopedClock({None: tick_clock.global_clock})
            )
            popped = self.nc._tile_sem_poison_stack.pop()
            assert popped is self._sem_poison

    f32 = mybir.dt.float32
    bf16 = mybir.dt.bfloat16
    AF = mybir.ActivationFunctionType

    nc = bacc.Bacc("TRN2")

    bf = mybir.dt.float32r if precision == "f32r" else bf16

    # L-blocks (kc>=dc, per dc in emission order dc=3,2,1,0) + A chunks,
    # all fused into one per-partition-contiguous tensor for a single
    # fat-packet DMA.  Column offsets precomputed here.
    DCS = [3, 2, 1, 0]
    lblk = {}
    col = 0
    for dc in DCS:
        for kc in range(dc, 4):
            lblk[(dc, kc)] = col
            col += 128
    acol = {}
    for kc in range(4):
        acol[kc] = col
        col += NAUG
    LWA_W = col  # 10*128 + 4*34 = 1416

    xt = nc.dram_tensor("xt", [128, 2, 4, 512], bf, kind="ExternalInput")
    lwa = nc.dram_tensor("lwa", [128, LWA_W], bf, kind="ExternalInput")
    # out[0:34, jt, :] = Y^T (32 scores + colsum + bcross); out[64, jt, :] =
    # sumsq.  Y and SQ share one [65, 512] PSUM tile per jt — the sumsq
    # ones-matmuls target partition 64 via tile_position=(0, 64) (output
    # base partition must be a multiple of 32) — so each jt ships with a
    # single copy + DMA.
    out_h = nc.dram_tensor("out", [65, 2, 512], f32, kind="ExternalOutput")

    ones_bf = nc.const_aps.tensor(1.0, [128, 1], bf16)

    with LeanTailTileContext(nc) as tc:
        with (
            tc.tile_pool(name="cst", bufs=1) as cst,
            tc.tile_pool(name="scr", bufs=2) as scr,
            tc.tile_pool(name="ps_u", bufs=4, space="PSUM") as ps_u,
            tc.tile_pool(name="ps_y", bufs=2, space="PSUM") as ps_y,
            tc.tile_pool(name="ps_w", bufs=1, space="PSUM") as ps_w,
        ):
            xt_sb = cst.tile([128, 2, 4, 512], bf, name="xt_sb", tag="xt")
            lwa_sb = cst.tile([128, LWA_W], bf, name="lwa_sb", tag="lwa")
            out_sb = cst.tile([65, 2, 512], f32, name="out_sb")

            # All input DMA on the Sync HWDGE ring: one ring at full rate
            # beats two shared ones, and the Scalar ring stalls ~1.5us
            # behind its activation-table load.  jt0's x first.
            nc.sync.dma_start(lwa_sb[:], lwa[:])
            nc.sync.dma_start(xt_sb[:, 0], xt[:, 0])
            nc.sync.dma_start(xt_sb[:, 1], xt[:, 1])

            # PE warm-up during the DMA fill: the HAM activity monitor only
            # un-throttles (1.2 -> 2.4 GHz) after ~3.4us of genuinely busy
            # PE; N=1 matmuls don't register, so stream N=256 ones off a
            # memset tile (baseline-style).
            if warmup:
                warm_sb = cst.tile([128, 256], bf16, name="warm_sb")
                nc.gpsimd.memset(warm_sb[:], 0.25)
                psw = ps_w.tile([128, 256], f32, name="psw", tag="psw")
                for _ in range(warmup):
                    nc.tensor.matmul(psw[:], warm_sb[:, 0:128], warm_sb[:],
                                     start=True, stop=True)

            for jt in range(2):
                # ---- U = L^T x (triangular: block dc needs kc>=dc) ----
                # dc=3 first (1 matmul) so its square lands early; the
                # sumsq ones-matmuls accumulate as squares become ready,
                # with Y before the last one so the PE never stalls.
                sq = {}
                for dc in DCS:
                    psu = ps_u.tile([128, 512], f32, name="psu", tag="psu")
                    kcs = list(range(dc, 4))
                    for ki, kc in enumerate(kcs):
                        nc.tensor.matmul(
                            psu[:],
                            lwa_sb[:, lblk[(dc, kc)]:lblk[(dc, kc)] + 128],
                            xt_sb[:, jt, kc, :],
                            start=(ki == 0),
                            stop=(ki == len(kcs) - 1),
                        )
                    # squared chunk (bf16; LN variance is error-tolerant)
                    sq_t = scr.tile([128, 512], bf16, name=f"sq{dc}",
                                    tag=f"sq{dc}")
                    nc.scalar.activation(sq_t[:], psu[:], AF.Square)
                    sq[dc] = sq_t

                py = ps_y.tile([65, 512], f32, name="py", tag="py")
                # sumsq partial sums into partition 64 as squares arrive
                for dc in [3, 2, 1]:
                    nc.tensor.matmul(py[64:65, :], ones_bf, sq[dc][:],
                                     start=(dc == 3), stop=False,
                                     tile_position=(0, 64))
                # ---- scores Y^T = A^T x into partitions 0..33 ----
                for kc in range(4):
                    nc.tensor.matmul(
                        py[0:NAUG, :],
                        lwa_sb[:, acol[kc]:acol[kc] + NAUG],
                        xt_sb[:, jt, kc, :],
                        start=(kc == 0),
                        stop=(kc == 3),
                    )
                # last sumsq chunk lands while Y streams
                nc.tensor.matmul(py[64:65, :], ones_bf, sq[0][:],
                                 start=False, stop=True,
                                 tile_position=(0, 64))

                nc.vector.tensor_copy(out_sb[:, jt, :], py[:])
                nc.sync.dma_start(out_h[:, jt, :], out_sb[:, jt, :])

    nc.compile()
    return nc


def _host_precompute(inputs):
    x = np.asarray(inputs["x"], np.float32)
    W = np.asarray(inputs["W_map"], np.float32)
    b_map = np.asarray(inputs["b_map"], np.float32)
    g = np.asarray(inputs["ln_g"], np.float32)
    lb = np.asarray(inputs["ln_b"], np.float32)
    Wq = np.asarray(inputs["Wq"], np.float32)
    bq = np.asarray(inputs["bq"], np.float32)
    Wk = np.asarray(inputs["Wk"], np.float32)
    bk = np.asarray(inputs["bk"], np.float32)
    tt = np.asarray(inputs["time_token"], np.float32)
    ct = np.asarray(inputs["cluster_token"], np.float32)

    spec = np.concatenate([tt, ct], 0)                      # [2, D]
    mu = spec.mean(-1, keepdims=True)
    var = ((spec - mu) ** 2).mean(-1, keepdims=True)
    hspec = ((spec - mu) / np.sqrt(var + EPS) * g + lb).reshape(2, H, DH)
    q = np.einsum("rhd,hde->rhe", hspec, Wq) + bq[None]
    qs = (q / np.sqrt(DH)).astype(np.float32)               # [2,H,DH]
    kspec = np.einsum("rhd,hde->rhe", hspec, Wk) + bk[None]
    s_spec = np.einsum("rhe,the->hrt", qs, kspec)           # [H,2,2]

    v = np.einsum("hde,rhe->hdr", Wk, qs)                   # [H,DH,2]
    V = np.zeros((D, 2 * H), np.float32)
    for h in range(H):
        V[64 * h:64 * h + 64, 2 * h] = v[h, :, 0]
        V[64 * h:64 * h + 64, 2 * h + 1] = v[h, :, 1]
    c0 = np.empty(2 * H, np.float32)
    for h in range(H):
        c0[2 * h] = qs[0, h] @ bk[h]
        c0[2 * h + 1] = qs[1, h] @ bk[h]

    Vg = g[:, None] * V
    # augmented score matrix: [Vg | ones | b_map]
    Vaug = np.concatenate(
        [Vg, np.ones((D, 1), np.float32), b_map[:, None]], 1)  # [D, 34]
    consts = dict(
        pg=Vg.sum(0),
        qb=(lb[:, None] * V).sum(0),
        bVg=(b_map[:, None] * Vg).sum(0),
        bmean=b_map.mean(),
        bsq=(b_map ** 2).sum(),
        s_spec=s_spec,
        c0=c0,
    )

    # collapse the linear part through W; factor the quadratic part
    W64 = W.astype(np.float64)
    A = (W64 @ Vaug.astype(np.float64)).astype(np.float32)  # [512, 34]
    L = np.linalg.cholesky(W64 @ W64.T).astype(np.float32)  # [512, 512]
    return x, A, L, consts


def kernel(**inputs):
    from concourse.bass_utils import run_bass_kernel_spmd

    x, A, L, consts = _host_precompute(inputs)

    key = (PRECISION, WARMUP_MMS)
    if key not in _PROG_CACHE:
        _PROG_CACHE[key] = _build_program(PRECISION, WARMUP_MMS)
    nc = _PROG_CACHE[key]

    cast = (lambda a: np.asarray(a, np.float32)) if PRECISION == "f32r" \
        else _bf16

    xf = x.reshape(B * N, IN_D)
    xpad = np.zeros((JTOT, IN_D), np.float32)
    xpad[:B * N] = xf

    # fused L-blocks (kc>=dc, dc order 3,2,1,0) + A chunks: [128, 1416]
    lwa_np = np.empty((128, 10 * 128 + 4 * NAUG), np.float32)
    col = 0
    for dc in [3, 2, 1, 0]:
        for kc in range(dc, 4):
            lwa_np[:, col:col + 128] = \
                L[kc * 128:(kc + 1) * 128, dc * 128:(dc + 1) * 128]
            col += 128
    for kc in range(4):
        lwa_np[:, col:col + NAUG] = A[kc * 128:(kc + 1) * 128, :]
        col += NAUG
    shared = {"lwa": np.ascontiguousarray(cast(lwa_np))}

    in_maps = []
    for c in range(NCORES):
        xT = np.ascontiguousarray(xpad[c * JPC:(c + 1) * JPC].T)  # [512,1024]
        m = dict(shared)
        # [512, 1024] -> [128p, 2jt, 4kc, 512]
        m["xt"] = np.ascontiguousarray(
            cast(xT).reshape(4, 128, 2, 512).transpose(1, 2, 0, 3))
        in_maps.append(m)

    trace = bool(int(os.environ.get("AS_TRACE", "0")))
    res = run_bass_kernel_spmd(nc, in_maps, list(range(NCORES)), trace=trace)
    global LAST_RESULT
    LAST_RESULT = res
    outs = [np.asarray(r["out"], np.float32) for r in res.results]

    return _epilogue(outs, consts)


def _epilogue(outs, consts):
    # outs: per-core [65, 2, 512]; [0:34,jt,:] = Y^T, [64,jt,:] = SQ
    yfull = np.concatenate(
        [o[0:NAUG].reshape(NAUG, JPC).T for o in outs], 0)[:B * N]
    SQ = np.concatenate([o[64].reshape(JPC) for o in outs], 0)[:B * N]
    Y = yfull[:, 0:32]
    colsum = yfull[:, 32]
    bcross = yfull[:, 33]

    mu = colsum / np.float32(D) + consts["bmean"]
    E2 = (SQ + 2.0 * bcross + consts["bsq"]) / np.float32(D)
    var = E2 - mu ** 2
    rstd = (1.0 / np.sqrt(var + EPS)).astype(np.float32)
    G = Y + consts["bVg"][None]
    sc = (rstd[:, None] * G
          - (rstd * mu)[:, None] * consts["pg"][None]
          + consts["qb"][None] + consts["c0"][None])
    sc = sc.reshape(B, N, H, 2).transpose(0, 2, 3, 1)       # [B,H,2,N]

    scores = np.empty((B, H, 2, S), np.float32)
    scores[:, :, :, 2:] = sc
    scores[:, :, :, 0:2] = consts["s_spec"][None]

    m = scores - scores.max(-1, keepdims=True)
    e = np.exp(m)
    attn = e / e.sum(-1, keepdims=True)
    mm = attn.mean(1)                                       # [B,2,S]
    return (np.ascontiguousarray(mm[:, 0, :]),
            np.ascontiguousarray(mm[:, 1, :]))


# revision 22
# speedup vs baseline: 1.5833x; 1.1434x over previous
"""Trainium2 Bass kernel for nn_AttentionSiphon.

Reference computes: tokens = x @ W_map + b_map; concat [time, cluster, tokens];
LayerNorm; per-head q/k projections; softmax(q k^T / sqrt(dh)); mean over heads;
returns rows 0 and 1 of the [B,S,S] head-mean attention.

Only attention rows 0/1 are returned, and their queries come from the
(batch-independent) time/cluster tokens, so per-head attention collapses to

  score[j, c=2h+r] = LN(token_j) . (Wk[h] @ q_r[h])   (+ constants)

The 34 score/stat columns are LINEAR in x:  Y = Vaug^T (W^T x^T) = A^T x^T
with A = W @ Vaug [512, 34] precomputed on host.  Only the LayerNorm
sum-of-squares is quadratic:  SQ_j = ||W^T x_j||^2 = x_j^T (W W^T) x_j
= ||L^T x_j||^2 with L = cholesky(W W^T) [512, 512].  So the device work per
core (1024 token columns) is U = L^T x (512-contraction, half the FLOPs of the
naive 1024-wide token projection), squares+reduce for SQ, and the tiny A^T x.
L is lower-triangular, so of the 4x4 grid of [128,128] contraction blocks only
the kc >= dc ones are nonzero: 10 matmuls per 512-column tile instead of 16.

Device output per core: [34, 2, 1024] f32 — [:,0,:] = Y^T, [0,1,:] = SQ.
The tiny softmax epilogue runs on host (identical to the previous scheme).
"""

import os
import sys

sys.path.insert(0, "/opt/trn_rl_repo")

import numpy as np
import ml_dtypes

B, N, IN_D = 4, 2046, 512
D, H, DH = 1024, 16, 64
S = N + 2
EPS = 1e-5
NCORES = 8
JPC = 1024            # padded rows per core
JTOT = NCORES * JPC   # 8192 (8184 real rows + 8 pad)
NAUG = 34             # 32 score cols + colsum + b_map cross
NC_OUT = NAUG + 1     # + sumsq row

# Precision scheme: "bf16" (fastest, ~1.6e-3 rel err),
# "f32r" (fp32-storage reduced-precision matmuls at bf16 PE speed, ~2e-4)
PRECISION = os.environ.get("AS_PRECISION", "bf16")
WARMUP_MMS = int(os.environ.get("AS_WARM", "17"))

_PROG_CACHE = {}
LAST_RESULT = None  # BassKernelResults of the most recent run (for test harness)


def _bf16(a):
    return np.asarray(a, np.float32).astype(ml_dtypes.bfloat16)


def _build_program(precision, warmup=None):
    if warmup is None:
        warmup = WARMUP_MMS
    import concourse.bacc as bacc
    import concourse.mybir as mybir
    from concourse import tile
    from concourse.tile import ScopedClock

    class LeanTailTileContext(tile.TileContext):
        """Skip the exit-path double all-engine barrier + per-sem clears.

        The kernel preamble (Bass.__init__, target_bir_lowering) already
        dma_reset+sem_clears the kernel sem range at the start of every
        execution, and this program has a single TileContext, so nothing
        downstream consumes the freed sems. The final Sync drain still
        waits on every proc (incl. DMA lanes), so outputs are complete
        before the instruction streams end.
        """

        def _drain_and_barrier(self, tick_clock, wait_clock):
            drain_inst = self.nc.sync.drain()
            wait_clock.add_sem_waits(
                drain_inst.ins, ScopedClock({None: tick_clock.global_clock})
            )
            popped = self.nc._tile_sem_poison_stack.pop()
            assert popped is self._sem_poison

    f32 = mybir.dt.float32
    bf16 = mybir.dt.bfloat16
    AF = mybir.ActivationFunctionType

    nc = bacc.Bacc("TRN2")

    bf = mybir.dt.float32r if precision == "f32r" else bf16

    # L-blocks (kc>=dc, per dc in emission order dc=3,2,1,0) + A chunks,
    # all fused into one per-partition-contiguous tensor for a single
    # fat-packet DMA.  Column offsets precomputed here.
    DCS = [3, 2, 1, 0]
    lblk = {}
    col = 0
    for dc in DCS:
        for kc in range(dc, 4):
            lblk[(dc, kc)] = col
            col += 128
    acol = {}
    for kc in range(4):
        acol[kc] = col
        col += NAUG
    LWA_W = 1536  # 10*128 + 4*34 = 1416, padded for alignment
    X0 = LWA_W    # xt jt0 kc-chunks live at X0 + kc*512 in in0

    # Inputs fused into two fat tensors so each partition row is one long
    # contiguous DRAM region (7KB / 4KB) — short rows starve the DMA
    # engines on descriptor fetches (measured 58% vs 100% engine busy).
    in0 = nc.dram_tensor("in0", [128, LWA_W + 2048], bf, kind="ExternalInput")
    in1 = nc.dram_tensor("in1", [128, 2048], bf, kind="ExternalInput")
    # out[0:34, jt, :] = Y^T (32 scores + colsum + bcross); out[64, jt, :] =
    # sumsq.  Y and SQ share one [65, 512] PSUM tile per jt — the sumsq
    # ones-matmuls target partition 64 via tile_position=(0, 64) (output
    # base partition must be a multiple of 32) — so each jt ships with a
    # single copy + DMA.
    out_h = nc.dram_tensor("out", [65, 2, 512], f32, kind="ExternalOutput")

    ones_bf = nc.const_aps.tensor(1.0, [128, 1], bf16)

    with LeanTailTileContext(nc) as tc:
        with (
            tc.tile_pool(name="cst", bufs=1) as cst,
            tc.tile_pool(name="scr", bufs=2) as scr,
            tc.tile_pool(name="ps_u", bufs=4, space="PSUM") as ps_u,
            tc.tile_pool(name="ps_y", bufs=2, space="PSUM") as ps_y,
            tc.tile_pool(name="ps_w", bufs=1, space="PSUM") as ps_w,
        ):
            in0_sb = cst.tile([128, LWA_W + 2048], bf, name="in0_sb",
                              tag="in0")
            in1_sb = cst.tile([128, 2048], bf, name="in1_sb", tag="in1")
            out_sb = cst.tile([65, 2, 512], f32, name="out_sb")

            def lwa_sl(c, w):
                return in0_sb[:, c:c + w]

            def xt_sl(jt, kc):
                if jt == 0:
                    return in0_sb[:, X0 + kc * 512:X0 + (kc + 1) * 512]
                return in1_sb[:, kc * 512:(kc + 1) * 512]

            # All input DMA on the Sync HWDGE ring: one ring at full rate
            # beats two shared ones, and the Scalar ring stalls ~1.5us
            # behind its activation-table load.  jt0's data (weights + x)
            # in the first transfer.
            nc.sync.dma_start(in0_sb[:], in0[:])
            nc.sync.dma_start(in1_sb[:], in1[:])

            # PE warm-up during the DMA fill: the HAM activity monitor only
            # un-throttles (1.2 -> 2.4 GHz) after ~3.4us of genuinely busy
            # PE; N=1 matmuls don't register, so stream N=256 ones off a
            # memset tile (baseline-style).
            if warmup:
                warm_sb = cst.tile([128, 256], bf16, name="warm_sb")
                nc.gpsimd.memset(warm_sb[:], 0.25)
                psw = ps_w.tile([128, 256], f32, name="psw", tag="psw")
                for _ in range(warmup):
                    nc.tensor.matmul(psw[:], warm_sb[:, 0:128], warm_sb[:],
                                     start=True, stop=True)

            for jt in range(2):
                # ---- U = L^T x (triangular: block dc needs kc>=dc) ----
                # dc=3 first (1 matmul) so its square lands early; the
                # sumsq ones-matmuls accumulate as squares become ready,
                # with Y before the last one so the PE never stalls.
                sq = {}
                for dc in DCS:
                    psu = ps_u.tile([128, 512], f32, name="psu", tag="psu")
                    kcs = list(range(dc, 4))
                    for ki, kc in enumerate(kcs):
                        nc.tensor.matmul(
                            psu[:],
                            lwa_sl(lblk[(dc, kc)], 128),
                            xt_sl(jt, kc),
                            start=(ki == 0),
                            stop=(ki == len(kcs) - 1),
                        )
                    # squared chunk (bf16; LN variance is error-tolerant)
                    sq_t = scr.tile([128, 512], bf16, name=f"sq{dc}",
                                    tag=f"sq{dc}")
                    nc.scalar.activation(sq_t[:], psu[:], AF.Square)
                    sq[dc] = sq_t

                py = ps_y.tile([65, 512], f32, name="py", tag="py")
                # sumsq partial sums into partition 64 as squares arrive
                for dc in [3, 2, 1]:
                    nc.tensor.matmul(py[64:65, :], ones_bf, sq[dc][:],
                                     start=(dc == 3), stop=False,
                                     tile_position=(0, 64))
                # ---- scores Y^T = A^T x into partitions 0..33 ----
                for kc in range(4):
                    nc.tensor.matmul(
                        py[0:NAUG, :],
                        lwa_sl(acol[kc], NAUG),
                        xt_sl(jt, kc),
                        start=(kc == 0),
                        stop=(kc == 3),
                    )
                # last sumsq chunk lands while Y streams
                nc.tensor.matmul(py[64:65, :], ones_bf, sq[0][:],
                                 start=False, stop=True,
                                 tile_position=(0, 64))

                nc.vector.tensor_copy(out_sb[:, jt, :], py[:])
                nc.sync.dma_start(out_h[:, jt, :], out_sb[:, jt, :])

    nc.compile()
    return nc


def _host_precompute(inputs):
    x = np.asarray(inputs["x"], np.float32)
    W = np.asarray(inputs["W_map"], np.float32)
    b_map = np.asarray(inputs["b_map"], np.float32)
    g = np.asarray(inputs["ln_g"], np.float32)
    lb = np.asarray(inputs["ln_b"], np.float32)
    Wq = np.asarray(inputs["Wq"], np.float32)
    bq = np.asarray(inputs["bq"], np.float32)
    Wk = np.asarray(inputs["Wk"], np.float32)
    bk = np.asarray(inputs["bk"], np.float32)
    tt = np.asarray(inputs["time_token"], np.float32)
    ct = np.asarray(inputs["cluster_token"], np.float32)

    spec = np.concatenate([tt, ct], 0)                      # [2, D]
    mu = spec.mean(-1, keepdims=True)
    var = ((spec - mu) ** 2).mean(-1, keepdims=True)
    hspec = ((spec - mu) / np.sqrt(var + EPS) * g + lb).reshape(2, H, DH)
    q = np.einsum("rhd,hde->rhe", hspec, Wq) + bq[None]
    qs = (q / np.sqrt(DH)).astype(np.float32)               # [2,H,DH]
    kspec = np.einsum("rhd,hde->rhe", hspec, Wk) + bk[None]
    s_spec = np.einsum("rhe,the->hrt", qs, kspec)           # [H,2,2]

    v = np.einsum("hde,rhe->hdr", Wk, qs)                   # [H,DH,2]
    V = np.zeros((D, 2 * H), np.float32)
    for h in range(H):
        V[64 * h:64 * h + 64, 2 * h] = v[h, :, 0]
        V[64 * h:64 * h + 64, 2 * h + 1] = v[h, :, 1]
    c0 = np.empty(2 * H, np.float32)
    for h in range(H):
        c0[2 * h] = qs[0, h] @ bk[h]
        c0[2 * h + 1] = qs[1, h] @ bk[h]

    Vg = g[:, None] * V
    # augmented score matrix: [Vg | ones | b_map]
    Vaug = np.concatenate(
        [Vg, np.ones((D, 1), np.float32), b_map[:, None]], 1)  # [D, 34]
    consts = dict(
        pg=Vg.sum(0),
        qb=(lb[:, None] * V).sum(0),
        bVg=(b_map[:, None] * Vg).sum(0),
        bmean=b_map.mean(),
        bsq=(b_map ** 2).sum(),
        s_spec=s_spec,
        c0=c0,
    )

    # collapse the linear part through W; factor the quadratic part
    W64 = W.astype(np.float64)
    A = (W64 @ Vaug.astype(np.float64)).astype(np.float32)  # [512, 34]
    L = np.linalg.cholesky(W64 @ W64.T).astype(np.float32)  # [512, 512]
    return x, A, L, consts


def kernel(**inputs):
    from concourse.bass_utils import run_bass_kernel_spmd

    x, A, L, consts = _host_precompute(inputs)

    key = (PRECISION, WARMUP_MMS)
    if key not in _PROG_CACHE:
        _PROG_CACHE[key] = _build_program(PRECISION, WARMUP_MMS)
    nc = _PROG_CACHE[key]

    cast = (lambda a: np.asarray(a, np.float32)) if PRECISION == "f32r" \
        else _bf16

    xf = x.reshape(B * N, IN_D)
    xpad = np.zeros((JTOT, IN_D), np.float32)
    xpad[:B * N] = xf

    # fused L-blocks (kc>=dc, dc order 3,2,1,0) + A chunks, zero-padded to
    # 1536 cols; x^T jt0 follows in the same tensor (fat DMA rows)
    LWA_W = 1536
    lwa_np = np.zeros((128, LWA_W), np.float32)
    col = 0
    for dc in [3, 2, 1, 0]:
        for kc in range(dc, 4):
            lwa_np[:, col:col + 128] = \
                L[kc * 128:(kc + 1) * 128, dc * 128:(dc + 1) * 128]
            col += 128
    for kc in range(4):
        lwa_np[:, col:col + NAUG] = A[kc * 128:(kc + 1) * 128, :]
        col += NAUG
    lwa_c = cast(lwa_np)

    in_maps = []
    for c in range(NCORES):
        xT = np.ascontiguousarray(xpad[c * JPC:(c + 1) * JPC].T)  # [512,1024]
        # [512, 1024] -> [128p, 2jt, 4kc, 512]
        xp = cast(xT).reshape(4, 128, 2, 512).transpose(1, 2, 0, 3)
        i0 = np.empty((128, LWA_W + 2048), lwa_c.dtype)
        i0[:, :LWA_W] = lwa_c
        i0[:, LWA_W:] = xp[:, 0].reshape(128, 2048)
        m = {"in0": np.ascontiguousarray(i0),
             "in1": np.ascontiguousarray(xp[:, 1].reshape(128, 2048))}
        in_maps.append(m)

    trace = bool(int(os.environ.get("AS_TRACE", "0")))
    res = run_bass_kernel_spmd(nc, in_maps, list(range(NCORES)), trace=trace)
    global LAST_RESULT
    LAST_RESULT = res
    outs = [np.asarray(r["out"], np.float32) for r in res.results]

    return _epilogue(outs, consts)


def _epilogue(outs, consts):
    # outs: per-core [65, 2, 512]; [0:34,jt,:] = Y^T, [64,jt,:] = SQ
    yfull = np.concatenate(
        [o[0:NAUG].reshape(NAUG, JPC).T for o in outs], 0)[:B * N]
    SQ = np.concatenate([o[64].reshape(JPC) for o in outs], 0)[:B * N]
    Y = yfull[:, 0:32]
    colsum = yfull[:, 32]
    bcross = yfull[:, 33]

    mu = colsum / np.float32(D) + consts["bmean"]
    E2 = (SQ + 2.0 * bcross + consts["bsq"]) / np.float32(D)
    var = E2 - mu ** 2
    rstd = (1.0 / np.sqrt(var + EPS)).astype(np.float32)
    G = Y + consts["bVg"][None]
    sc = (rstd[:, None] * G
          - (rstd * mu)[:, None] * consts["pg"][None]
          + consts["qb"][None] + consts["c0"][None])
    sc = sc.reshape(B, N, H, 2).transpose(0, 2, 3, 1)       # [B,H,2,N]

    scores = np.empty((B, H, 2, S), np.float32)
    scores[:, :, :, 2:] = sc
    scores[:, :, :, 0:2] = consts["s_spec"][None]

    m = scores - scores.max(-1, keepdims=True)
    e = np.exp(m)
    attn = e / e.sum(-1, keepdims=True)
    mm = attn.mean(1)                                       # [B,2,S]
    return (np.ascontiguousarray(mm[:, 0, :]),
            np.ascontiguousarray(mm[:, 1, :]))


# revision 30
# speedup vs baseline: 1.6312x; 1.0302x over previous
"""Trainium2 Bass kernel for nn_AttentionSiphon.

Reference computes: tokens = x @ W_map + b_map; concat [time, cluster, tokens];
LayerNorm; per-head q/k projections; softmax(q k^T / sqrt(dh)); mean over heads;
returns rows 0 and 1 of the [B,S,S] head-mean attention.

Only attention rows 0/1 are returned, and their queries come from the
(batch-independent) time/cluster tokens, so per-head attention collapses to

  score[j, c=2h+r] = LN(token_j) . (Wk[h] @ q_r[h])   (+ constants)

The 34 score/stat columns are LINEAR in x:  Y = Vaug^T (W^T x^T) = A^T x^T
with A = W @ Vaug [512, 34] precomputed on host.  Only the LayerNorm
sum-of-squares is quadratic:  SQ_j = ||W^T x_j||^2 = x_j^T (W W^T) x_j
= ||L^T x_j||^2 with L = cholesky(W W^T) [512, 512].  So the device work per
core (1024 token columns) is U = L^T x (512-contraction, half the FLOPs of the
naive 1024-wide token projection), squares+reduce for SQ, and the tiny A^T x.
L is lower-triangular, so of the 4x4 grid of [128,128] contraction blocks only
the kc >= dc ones are nonzero: 10 matmuls per 512-column tile instead of 16.

Device output per core: [34, 2, 1024] f32 — [:,0,:] = Y^T, [0,1,:] = SQ.
The tiny softmax epilogue runs on host (identical to the previous scheme).
"""

import os
import sys

sys.path.insert(0, "/opt/trn_rl_repo")

import numpy as np
import ml_dtypes

B, N, IN_D = 4, 2046, 512
D, H, DH = 1024, 16, 64
S = N + 2
EPS = 1e-5
NCORES = 8
JPC = 1024            # padded rows per core
JTOT = NCORES * JPC   # 8192 (8184 real rows + 8 pad)
NAUG = 34             # 32 score cols + colsum + b_map cross
NC_OUT = NAUG + 1     # + sumsq row

# Precision scheme: "bf16" (fastest, ~1.6e-3 rel err),
# "f32r" (fp32-storage reduced-precision matmuls at bf16 PE speed, ~2e-4)
PRECISION = os.environ.get("AS_PRECISION", "bf16")
WARMUP_MMS = int(os.environ.get("AS_WARM", "17"))

_PROG_CACHE = {}
LAST_RESULT = None  # BassKernelResults of the most recent run (for test harness)


def _bf16(a):
    return np.asarray(a, np.float32).astype(ml_dtypes.bfloat16)


def _build_program(precision, warmup=None):
    if warmup is None:
        warmup = WARMUP_MMS
    import concourse.bacc as bacc
    import concourse.mybir as mybir
    from concourse import tile
    from concourse.tile import ScopedClock

    class LeanTailTileContext(tile.TileContext):
        """Skip the exit-path double all-engine barrier + per-sem clears.

        The kernel preamble (Bass.__init__, target_bir_lowering) already
        dma_reset+sem_clears the kernel sem range at the start of every
        execution, and this program has a single TileContext, so nothing
        downstream consumes the freed sems. The final Sync drain still
        waits on every proc (incl. DMA lanes), so outputs are complete
        before the instruction streams end.
        """

        def _drain_and_barrier(self, tick_clock, wait_clock):
            drain_inst = self.nc.sync.drain()
            wait_clock.add_sem_waits(
                drain_inst.ins, ScopedClock({None: tick_clock.global_clock})
            )
            popped = self.nc._tile_sem_poison_stack.pop()
            assert popped is self._sem_poison

    f32 = mybir.dt.float32
    bf16 = mybir.dt.bfloat16
    AF = mybir.ActivationFunctionType

    nc = bacc.Bacc("TRN2")

    bf = mybir.dt.float32r if precision == "f32r" else bf16

    # L-blocks (kc>=dc, per dc in emission order dc=3,2,1,0) + A chunks,
    # all fused into one per-partition-contiguous tensor for a single
    # fat-packet DMA.  Column offsets precomputed here.
    DCS = [3, 2, 1, 0]
    lblk = {}
    col = 0
    for dc in DCS:
        for kc in range(dc, 4):
            lblk[(dc, kc)] = col
            col += 128
    acol = {}
    for kc in range(4):
        acol[kc] = col
        col += 32
    LWA_W = 1536  # 10*128 + 4*32 = 1408, padded for alignment
    X0 = LWA_W    # xt jt0 kc-chunks live at X0 + kc*512 in in0

    # Inputs fused into two fat tensors so each partition row is one long
    # contiguous DRAM region (7KB / 4KB) — short rows starve the DMA
    # engines on descriptor fetches (measured 58% vs 100% engine busy).
    in0 = nc.dram_tensor("in0", [128, LWA_W + 2048], bf, kind="ExternalInput")
    in1 = nc.dram_tensor("in1", [128, 2048], bf, kind="ExternalInput")
    # out[0:32, jt, :] = Y^T (32 scores); out[32, jt, :] = sumsq.  Y and SQ
    # share one [33, 512] PSUM tile per jt — the sumsq ones-matmuls target
    # partition 32 via tile_position=(0, 32) — so each jt needs a single
    # PSUM->SBUF copy, and one fat final DMA ships both jt halves.
    out_h = nc.dram_tensor("out", [33, 2, 512], f32, kind="ExternalOutput")

    ones_bf = nc.const_aps.tensor(1.0, [128, 1], bf16)

    with LeanTailTileContext(nc) as tc:
        with (
            tc.tile_pool(name="cst", bufs=1) as cst,
            tc.tile_pool(name="scr", bufs=2) as scr,
            tc.tile_pool(name="ps_u", bufs=4, space="PSUM") as ps_u,
            tc.tile_pool(name="ps_y", bufs=2, space="PSUM") as ps_y,
            tc.tile_pool(name="ps_w", bufs=1, space="PSUM") as ps_w,
        ):
            in0_sb = cst.tile([128, LWA_W + 2048], bf, name="in0_sb",
                              tag="in0")
            in1_sb = cst.tile([128, 2048], bf, name="in1_sb", tag="in1")
            out_sb = cst.tile([33, 2, 512], f32, name="out_sb")

            def lwa_sl(c, w):
                return in0_sb[:, c:c + w]

            def xt_sl(jt, kc):
                if jt == 0:
                    return in0_sb[:, X0 + kc * 512:X0 + (kc + 1) * 512]
                return in1_sb[:, kc * 512:(kc + 1) * 512]

            # All input DMA on the Sync HWDGE ring: one ring at full rate
            # beats two shared ones, and the Scalar ring stalls ~1.5us
            # behind its activation-table load.  jt0's data (weights + x)
            # in the first transfer.
            nc.sync.dma_start(in0_sb[:], in0[:])
            nc.sync.dma_start(in1_sb[:], in1[:])

            # PE warm-up during the DMA fill: the HAM activity monitor only
            # un-throttles (1.2 -> 2.4 GHz) after ~3.4us of genuinely busy
            # PE; N=1 matmuls don't register, so stream N=256 ones off a
            # memset tile (baseline-style).
            if warmup:
                warm_sb = cst.tile([128, 256], bf16, name="warm_sb")
                nc.gpsimd.memset(warm_sb[:], 0.25)
                psw = ps_w.tile([128, 256], f32, name="psw", tag="psw")
                for _ in range(warmup):
                    nc.tensor.matmul(psw[:], warm_sb[:, 0:128], warm_sb[:],
                                     start=True, stop=True)

            for jt in range(2):
                # ---- U = L^T x (triangular: block dc needs kc>=dc) ----
                # dc=3 first (1 matmul) so its square lands early; the
                # sumsq ones-matmuls accumulate as squares become ready,
                # with Y before the last one so the PE never stalls.
                sq = {}
                for dc in DCS:
                    psu = ps_u.tile([128, 512], f32, name="psu", tag="psu")
                    kcs = list(range(dc, 4))
                    for ki, kc in enumerate(kcs):
                        nc.tensor.matmul(
                            psu[:],
                            lwa_sl(lblk[(dc, kc)], 128),
                            xt_sl(jt, kc),
                            start=(ki == 0),
                            stop=(ki == len(kcs) - 1),
                        )
                    # squared chunk (bf16; LN variance is error-tolerant)
                    sq_t = scr.tile([128, 512], bf16, name=f"sq{dc}",
                                    tag=f"sq{dc}")
                    nc.scalar.activation(sq_t[:], psu[:], AF.Square)
                    sq[dc] = sq_t

                py = ps_y.tile([33, 512], f32, name="py", tag="py")
                # sumsq partial sums into partition 32 as squares arrive
                for dc in [3, 2, 1]:
                    nc.tensor.matmul(py[32:33, :], ones_bf, sq[dc][:],
                                     start=(dc == 3), stop=False,
                                     tile_position=(0, 32))
                # ---- scores Y^T = A^T x into partitions 0..31 ----
                for kc in range(4):
                    nc.tensor.matmul(
                        py[0:32, :],
                        lwa_sl(acol[kc], 32),
                        xt_sl(jt, kc),
                        start=(kc == 0),
                        stop=(kc == 3),
                    )
                # last sumsq chunk lands while Y streams
                nc.tensor.matmul(py[32:33, :], ones_bf, sq[0][:],
                                 start=False, stop=True,
                                 tile_position=(0, 32))

                nc.vector.tensor_copy(out_sb[:, jt, :], py[:])

            # one fat-row DMA for both halves (thin per-jt slices starve
            # the DMA engines; the jt0 half just waits for jt1's copy)
            nc.sync.dma_start(out_h[:], out_sb[:])

    nc.compile()
    return nc


def _host_precompute(inputs):
    x = np.asarray(inputs["x"], np.float32)
    W = np.asarray(inputs["W_map"], np.float32)
    b_map = np.asarray(inputs["b_map"], np.float32)
    g = np.asarray(inputs["ln_g"], np.float32)
    lb = np.asarray(inputs["ln_b"], np.float32)
    Wq = np.asarray(inputs["Wq"], np.float32)
    bq = np.asarray(inputs["bq"], np.float32)
    Wk = np.asarray(inputs["Wk"], np.float32)
    bk = np.asarray(inputs["bk"], np.float32)
    tt = np.asarray(inputs["time_token"], np.float32)
    ct = np.asarray(inputs["cluster_token"], np.float32)

    spec = np.concatenate([tt, ct], 0)                      # [2, D]
    mu = spec.mean(-1, keepdims=True)
    var = ((spec - mu) ** 2).mean(-1, keepdims=True)
    hspec = ((spec - mu) / np.sqrt(var + EPS) * g + lb).reshape(2, H, DH)
    q = np.einsum("rhd,hde->rhe", hspec, Wq) + bq[None]
    qs = (q / np.sqrt(DH)).astype(np.float32)               # [2,H,DH]
    kspec = np.einsum("rhd,hde->rhe", hspec, Wk) + bk[None]
    s_spec = np.einsum("rhe,the->hrt", qs, kspec)           # [H,2,2]

    v = np.einsum("hde,rhe->hdr", Wk, qs)                   # [H,DH,2]
    V = np.zeros((D, 2 * H), np.float32)
    for h in range(H):
        V[64 * h:64 * h + 64, 2 * h] = v[h, :, 0]
        V[64 * h:64 * h + 64, 2 * h + 1] = v[h, :, 1]
    c0 = np.empty(2 * H, np.float32)
    for h in range(H):
        c0[2 * h] = qs[0, h] @ bk[h]
        c0[2 * h + 1] = qs[1, h] @ bk[h]

    Vg = g[:, None] * V
    consts = dict(
        pg=Vg.sum(0),
        qb=(lb[:, None] * V).sum(0),
        bVg=(b_map[:, None] * Vg).sum(0),
        bmean=b_map.mean(),
        bsq=(b_map ** 2).sum(),
        s_spec=s_spec,
        c0=c0,
        # colsum/bcross are linear in x with tiny [512] maps — cheaper and
        # more accurate on host than as extra device score columns
        wc=(W @ np.ones(D, np.float32)).astype(np.float32),
        bc=(W @ b_map).astype(np.float32),
    )

    # collapse the linear part through W; factor the quadratic part
    W64 = W.astype(np.float64)
    A = (W64 @ Vg.astype(np.float64)).astype(np.float32)    # [512, 32]
    L = np.linalg.cholesky(W64 @ W64.T).astype(np.float32)  # [512, 512]
    return x, A, L, consts


def kernel(**inputs):
    from concourse.bass_utils import run_bass_kernel_spmd

    x, A, L, consts = _host_precompute(inputs)

    key = (PRECISION, WARMUP_MMS)
    if key not in _PROG_CACHE:
        _PROG_CACHE[key] = _build_program(PRECISION, WARMUP_MMS)
    nc = _PROG_CACHE[key]

    cast = (lambda a: np.asarray(a, np.float32)) if PRECISION == "f32r" \
        else _bf16

    xf = x.reshape(B * N, IN_D)
    xpad = np.zeros((JTOT, IN_D), np.float32)
    xpad[:B * N] = xf

    # fused L-blocks (kc>=dc, dc order 3,2,1,0) + A chunks, zero-padded to
    # 1536 cols; x^T jt0 follows in the same tensor (fat DMA rows)
    LWA_W = 1536
    lwa_np = np.zeros((128, LWA_W), np.float32)
    col = 0
    for dc in [3, 2, 1, 0]:
        for kc in range(dc, 4):
            lwa_np[:, col:col + 128] = \
                L[kc * 128:(kc + 1) * 128, dc * 128:(dc + 1) * 128]
            col += 128
    for kc in range(4):
        lwa_np[:, col:col + 32] = A[kc * 128:(kc + 1) * 128, :]
        col += 32
    lwa_c = cast(lwa_np)

    in_maps = []
    for c in range(NCORES):
        xT = np.ascontiguousarray(xpad[c * JPC:(c + 1) * JPC].T)  # [512,1024]
        # [512, 1024] -> [128p, 2jt, 4kc, 512]
        xp = cast(xT).reshape(4, 128, 2, 512).transpose(1, 2, 0, 3)
        i0 = np.empty((128, LWA_W + 2048), lwa_c.dtype)
        i0[:, :LWA_W] = lwa_c
        i0[:, LWA_W:] = xp[:, 0].reshape(128, 2048)
        m = {"in0": np.ascontiguousarray(i0),
             "in1": np.ascontiguousarray(xp[:, 1].reshape(128, 2048))}
        in_maps.append(m)

    trace = bool(int(os.environ.get("AS_TRACE", "0")))
    res = run_bass_kernel_spmd(nc, in_maps, list(range(NCORES)), trace=trace)
    global LAST_RESULT
    LAST_RESULT = res
    outs = [np.asarray(r["out"], np.float32) for r in res.results]

    colsum = (xf @ consts["wc"]).astype(np.float32)
    bcross = (xf @ consts["bc"]).astype(np.float32)
    return _epilogue(outs, consts, colsum, bcross)


def _epilogue(outs, consts, colsum, bcross):
    # outs: per-core [33, 2, 512]; [0:32,jt,:] = Y^T, [32,jt,:] = SQ
    Y = np.concatenate(
        [o[0:32].reshape(32, JPC).T for o in outs], 0)[:B * N]
    SQ = np.concatenate([o[32].reshape(JPC) for o in outs], 0)[:B * N]

    mu = colsum / np.float32(D) + consts["bmean"]
    E2 = (SQ + 2.0 * bcross + consts["bsq"]) / np.float32(D)
    var = E2 - mu ** 2
    rstd = (1.0 / np.sqrt(var + EPS)).astype(np.float32)
    G = Y + consts["bVg"][None]
    sc = (rstd[:, None] * G
          - (rstd * mu)[:, None] * consts["pg"][None]
          + consts["qb"][None] + consts["c0"][None])
    sc = sc.reshape(B, N, H, 2).transpose(0, 2, 3, 1)       # [B,H,2,N]

    scores = np.empty((B, H, 2, S), np.float32)
    scores[:, :, :, 2:] = sc
    scores[:, :, :, 0:2] = consts["s_spec"][None]

    m = scores - scores.max(-1, keepdims=True)
    e = np.exp(m)
    attn = e / e.sum(-1, keepdims=True)
    mm = attn.mean(1)                                       # [B,2,S]
    return (np.ascontiguousarray(mm[:, 0, :]),
            np.ascontiguousarray(mm[:, 1, :]))
